# revision 2
# baseline (speedup 1.0000x reference)
"""GroupSort (pairwise channel sort) Trainium2 Bass kernel.

out[:, 2k]   = min(x[:, 2k], x[:, 2k+1])
out[:, 2k+1] = max(x[:, 2k], x[:, 2k+1])

x: [32, 512, 56, 56] f32.  Batch-sharded across 8 NeuronCores (4 per core).
Per core the shard [4, 512, 56, 56] is viewed as [1024, 6272]: each row is
one (batch, channel-pair) - first 3136 cols = even channel's H*W pixels,
last 3136 = odd channel's.  Memory-bound: 25.7 MB in + 25.7 MB out per core.

Compute is replicated bit-exactly from the reference:
  z = relu(xe - xo); out_e = xe - z; out_o = xo + z
spread across engines so the DVE doesn't throttle SDMA engine 15:
  DVE: v = xe - xo, out_e = xe - z (in-place)
  ACT: z = relu(v) (in-place)
  Pool: out_o = xo + z (in-place)
Outputs overwrite the input tile, so each tile needs one full-row store
(25 KiB descriptors) instead of two half-row ones.
"""

import os
import sys

import numpy as np

sys.path.insert(0, "/opt/trn_rl_repo")

import concourse.tile as tile
from concourse import bacc, mybir
from concourse.bass_utils import run_bass_kernel_spmd

def _install_trace_shim():
    """The image's antenv package lacks axon_hooks, which
    run_bass_kernel_spmd imports for trace=True. Install the same
    ctypes-based NTFF hook trn_boot would have registered, and keep
    profile artifacts local instead of uploading to a bucket."""
    try:
        import types as _types

        from concourse import bass_utils as _bu

        _bu.upload_artifacts = lambda tmpdir: tmpdir
        if "antenv.axon_hooks" not in sys.modules:
            from trn_agent_boot.trn_boot import _ntff_profile_via_ctypes

            _hook = _ntff_profile_via_ctypes("/opt/axon/libaxon_pjrt.so")
            _mod = _types.ModuleType("antenv.axon_hooks")
            _mod.get_axon_ntff_profile_hook = lambda: _hook
            _mod.set_axon_ntff_profile_hook = lambda h: None
            sys.modules["antenv.axon_hooks"] = _mod
    except Exception:
        pass


N_CORES = 8
B, C, H, W = 32, 512, 56, 56
HW = H * W  # 3136
B_PER = B // N_CORES  # 4
ROWS = B_PER * C // 2  # 1024 pair-rows per core
COLS = 2 * HW  # 6272
P = 128
N_TILES = ROWS // P  # 8

_cache = {}


def _build_nc():
    nc = bacc.Bacc(
        "TRN2", debug=False, num_devices=N_CORES, enable_partition_id=False
    )
    x = nc.dram_tensor("x", [ROWS, COLS], mybir.dt.float32, kind="ExternalInput").ap()
    o = nc.dram_tensor(
        "out", [ROWS, COLS], mybir.dt.float32, kind="ExternalOutput"
    ).ap()

    relu = mybir.ActivationFunctionType.Relu

    with tile.TileContext(nc, num_cores=N_CORES) as tc:
        with (
            tc.tile_pool(name="inp", bufs=5) as inp,
            tc.tile_pool(name="zp", bufs=3) as zp,
        ):
            tiles = []
            for t in range(N_TILES):
                r = t * P
                it = inp.tile([P, COLS], mybir.dt.float32)
                nc.sync.dma_start(out=it[:], in_=x[r : r + P, :])
                tiles.append(it)
                zt = zp.tile([P, HW], mybir.dt.float32)
                # v = xe - xo
                nc.vector.tensor_sub(zt[:], it[:, 0:HW], it[:, HW:COLS])
                # z = relu(v) in-place on the scalar (ACT) engine
                nc.scalar.activation(zt[:], zt[:], relu)
                # out_e = xe - z, in-place over xe (DVE)
                nc.vector.tensor_sub(it[:, 0:HW], it[:, 0:HW], zt[:])
                # out_o = xo + z, in-place over xo (Pool/GpSimd)
                nc.gpsimd.tensor_add(it[:, HW:COLS], it[:, HW:COLS], zt[:])
                # one full-row store (25 KiB contiguous per partition)
                nc.sync.dma_start(out=o[r : r + P, :], in_=it[:])
    nc.compile()
    return nc


def _get_nc():
    if "nc" not in _cache:
        _cache["nc"] = _build_nc()
    return _cache["nc"]


def kernel(
    x: np.ndarray,
    _trace: bool = False,
    _tmpdir: str | None = None,
    _trace_cores: list | None = None,
):
    assert x.shape == (B, C, H, W), x.shape
    x = np.ascontiguousarray(x, dtype=np.float32)
    shards = x.reshape(N_CORES, ROWS, COLS)
    in_maps = [{"x": shards[i]} for i in range(N_CORES)]

    nc = _get_nc()
    if _trace:
        _install_trace_shim()
        os.environ.pop("BASS_NEVER_TRACE", None)
    else:
        # run_bass_kernel_spmd also enables tracing when BASS_TRACE is set
        # in the environment; keep the grading path deterministic.
        os.environ["BASS_NEVER_TRACE"] = "1"
    res = run_bass_kernel_spmd(
        nc,
        in_maps,
        list(range(N_CORES)),
        trace=_trace,
        tmpdir=_tmpdir,
        trace_cores=_trace_cores,
    )
    out = np.empty((N_CORES, ROWS, COLS), dtype=np.float32)
    for i in range(N_CORES):
        out[i] = res.results[i]["out"]
    if _trace:
        kernel.last_exec_time_ns = res.exec_time_ns
        kernel.last_results = res
    return out.reshape(B, C, H, W)


if __name__ == "__main__":
    rng = np.random.default_rng(0)
    xt = rng.standard_normal((B, C, H, W), dtype=np.float32)
    yt = kernel(xt)
    xe, xo = xt[:, 0::2], xt[:, 1::2]
    z = np.maximum(xe - xo, 0)
    exp = np.empty_like(xt)
    exp[:, 0::2] = xe - z
    exp[:, 1::2] = xo + z
    err = np.abs(yt - exp).max()
    print("absmax err:", err)
